# revision 28
# baseline (speedup 1.0000x reference)
"""Trainium2 Bass kernel for a quantized ResNet BasicBlock:

    out = relu(bn2(qconv2(relu(bn1(qconv1(x))))) + x)

where qconv = 3x3 conv (stride 1, pad 1) on 8-bit symmetric per-tensor
quantized activations/weights (wage-style, forward pass only), and bn is
training-mode BatchNorm2d (batch statistics over N,H,W).

Strategy (8 NeuronCores, data-parallel over batch):
  * Each core gets B/8 samples. Weights/BN params replicated.
  * Quantized values round(v/s*127) are integers in [-127,127] — exact in
    bfloat16 — so each 3x3 conv runs as 9 accumulated bf16 128x128 matmuls
    per output chunk (channels on the partition dim, shifted windows over a
    zero-padded spatial free dim), accumulating exactly in f32 PSUM. The
    (s_in*s_w/127^2) scale is folded into the BN affine transform.
  * All cross-core exchanges are AllGather (cheap mesh forwarding) plus a
    local PE-transpose + DVE reduce; AllReduce (slow mesh compute) is
    avoided.  Collective payloads are staged through a PE pre-transpose so
    each staging DMA is 1-3 descriptors instead of 128 tiny ones.
  * x is loaded with half-sample DMAs alternating between the two HWDGE
    queues (sync/scalar) — descriptor dispatch is the load bottleneck —
    with the local absmax pipelined per half.
  * Engine balance in the convs: PE does matmuls (~59us/conv, the roofline),
    ACT does PSUM->SBUF copies (+channel-sum accumulation) and quantize
    pass 1, DVE does square+sumsq, channel-max and quantize pass 2.
  * gamma=1 => the BN scale A is positive, so the post-BN1-relu quant
    scale needs only channel maxima of the raw conv output (no minima).
  * round-to-nearest-even via the f32 magic-number trick (+1.5*2^23 then
    subtract), matching jnp.round.
  * Output is stored/DMA'd as bf16 (rel-err budget 2e-2; bf16 adds ~2e-3).
"""

import numpy as np

import concourse.bass as bass
import concourse.bacc as bacc
import concourse.mybir as mybir
import concourse.tile as tile
from concourse import bass_isa
from concourse import bass_utils
from concourse.bass_interp import get_hw_module

f32 = mybir.dt.float32
bf16 = mybir.dt.bfloat16
AF = mybir.ActivationFunctionType
OP = mybir.AluOpType
AX = mybir.AxisListType

N_CORES = 8
MAGIC = 12582912.0  # 1.5 * 2^23: (t + MAGIC) - MAGIC == rint(t) for |t| < 2^22
EPS = 1e-5
QMAX = 127.0


def build_module(B=32, C=128, H=56, W=56, n_cores=N_CORES, rows_per_chunk=8):
    npc = B // n_cores          # samples per core
    HWl = H * W
    HH = HWl // 2
    WP = W + 2                  # padded row length
    PADLEN = (H + 2) * WP       # padded image size
    XKLEN = PADLEN + 2          # +1 guard element at each end
    RPC = rows_per_chunk
    assert H % RPC == 0
    NCH = H // RPC              # chunks (row groups) per sample
    CF = RPC * WP               # matmul free size per chunk
    assert CF <= 512
    M = B * HWl                 # BN normalization count (global batch)
    K9 = 9 * C

    nc = bacc.Bacc("TRN2", target_bir_lowering=False, debug=False,
                   num_devices=n_cores)

    x_d = nc.dram_tensor("x", [npc, C, HWl], f32, kind="ExternalInput")
    xh_d = nc.dram_tensor("xh", [npc, C, HWl], bf16, kind="ExternalInput")
    w1_d = nc.dram_tensor("w1t", [C, K9], f32, kind="ExternalInput")
    w2_d = nc.dram_tensor("w2t", [C, K9], f32, kind="ExternalInput")
    par_d = nc.dram_tensor("params", [C, 4], f32, kind="ExternalInput")
    eye_d = nc.dram_tensor("eyec", [C, C], f32, kind="ExternalInput")
    out_d = nc.dram_tensor("out", [npc, C, HWl], bf16, kind="ExternalOutput")

    groups = [list(range(n_cores))]

    with tile.TileContext(nc) as tc:
        with (
            tc.tile_pool(name="const", bufs=1) as constp,
            tc.tile_pool(name="xs", bufs=1) as xsp,
            tc.tile_pool(name="act", bufs=1) as actp,
            tc.tile_pool(name="z", bufs=1) as zp,
            tc.tile_pool(name="small", bufs=1) as smallp,
            tc.tile_pool(name="sq", bufs=4) as sqp,
            tc.tile_pool(name="psum", bufs=8, space="PSUM") as psump,
            tc.tile_pool(name="dram", bufs=1, space="DRAM") as dramp,
        ):
            def stile(tag, cols=1):
                return smallp.tile([C, cols], f32, tag=tag, name=tag)

            # ---------------- absmax from the bf16 copy of x ---------------
            # the quant scale feeds only scale-invariant BN math, so a bf16
            # absmax (half the HBM traffic of f32 — the pre-barrier critical
            # path) is accurate far beyond what the output needs.
            xhs = []
            xmaxs = stile("xmaxs", npc)
            for n in range(npc):
                th = actp.tile([C, XKLEN], bf16, tag=f"act{n}", name=f"xh{n}")
                eng = nc.sync if n % 2 == 0 else nc.scalar
                eng.dma_start(th[:, 0:HWl], xh_d[n])
                nc.vector.tensor_reduce(out=xmaxs[:, n:n + 1],
                                        in_=th[:, 0:HWl], axis=AX.X, op=OP.max,
                                        apply_absolute_value=True)
                xhs.append(th)
            # identity for PE transposes: on sync after the bf16 halves
            # (needed for the s_x staging transpose at ~22us)
            eye_sb = smallp.tile([C, C], f32, tag="eyec", name="eyec")
            nc.sync.dma_start(eye_sb[:], eye_d[:])
            eye8 = eye_sb[0:n_cores, 0:n_cores]

            xmax = stile("xmax")
            nc.vector.tensor_reduce(out=xmax[:], in_=xmaxs[:], axis=AX.X,
                                    op=OP.max)
            # transpose [C,1] -> [1,C] so the staging DMA is ONE descriptor
            tpin = psump.tile([1, C], f32, tag="ps", name="tpin")
            nc.tensor.transpose(tpin[:], xmax[:], eye_sb[:])
            xrow = smallp.tile([1, C], f32, tag="xrow", name="xrow")
            nc.scalar.activation(out=xrow[:], in_=tpin[:], func=AF.Copy)
            # AllGather per-core per-channel maxima, reduce locally.
            # Stage + readback on sync — free once the bf16 halves land
            # (f32 x runs on scalar; SWDGE loses arbitration to busy HWDGE).
            ccx_i = dramp.tile([1, C], f32, tag="ccx_i", name="ccx_i")
            ccx_o = dramp.tile([n_cores, C], f32, tag="ccx_o", name="ccx_o")
            nc.gpsimd.dma_start(ccx_i[:], xrow[:])
            nc.gpsimd.collective_compute("AllGather", OP.bypass,
                                         replica_groups=groups,
                                         ins=[ccx_i[:].opt()],
                                         outs=[ccx_o[:].opt()])

            magic_t = stile("magic")
            nc.gpsimd.memset(magic_t[:], MAGIC)
            eps_t = stile("eps")
            nc.gpsimd.memset(eps_t[:], EPS)
            par_sb = stile("params", 4)
            nc.gpsimd.dma_start(par_sb[:], par_d[:])
            gamma1, beta1 = par_sb[:, 0:1], par_sb[:, 1:2]
            gamma2, beta2 = par_sb[:, 2:3], par_sb[:, 3:4]
            # preload the sqrt-capable activation table off the critical
            # path (a table switch at BN1 costs 1.5us otherwise)
            sqwarm = stile("sqwarm")
            nc.scalar.activation(out=sqwarm[:], in_=eps_t[:], func=AF.Sqrt,
                                 bias=eps_t[:], scale=1.0)

            # ---------------- x (f32): load after the bf16 copy ------------
            # parked past the s_x staging DMA (tile_wait_until) so its 512
            # descriptors don't bury the 1-descriptor stage in the queues
            xs = []
            for n in range(npc):
                t = xsp.tile([C, HWl], f32, tag=f"xs{n}", name=f"xs{n}")
                eng = nc.sync if n % 2 == 0 else nc.scalar
                eng.dma_start(t[:], x_d[n])
                xs.append(t)

            # ---------------- weights: load + quantize to integer bf16 ----
            wk = []     # bf16 integer lhsT weights [C, 9*C]
            wmaxg = []  # replicated per-tensor absmax [C,1]
            for j, w_d in enumerate((w1_d, w2_d)):
                wsb = constp.tile([C, K9], f32, tag=f"wsb{j}", name=f"wsb{j}")
                eng = nc.sync if j == 0 else nc.scalar
                eng.dma_start(wsb[:], w_d[:])
                wm = stile(f"wmax{j}")
                nc.vector.tensor_reduce(out=wm[:], in_=wsb[:], axis=AX.X,
                                        op=OP.max, apply_absolute_value=True)
                wmr = stile(f"wmaxr{j}")
                nc.gpsimd.partition_all_reduce(wmr[:], wm[:], channels=C,
                                               reduce_op=bass_isa.ReduceOp.max)
                wrec = stile(f"wrec{j}")
                nc.vector.reciprocal(wrec[:], wmr[:])
                cw = stile(f"cw{j}")
                nc.vector.tensor_scalar_mul(cw[:], wrec[:], QMAX)
                wtmp = constp.tile([C, K9], f32, tag=f"wtmp{j}", name=f"wtmp{j}")
                nc.scalar.activation(out=wtmp[:], in_=wsb[:], func=AF.Identity,
                                     bias=magic_t[:], scale=cw[:])
                wq = constp.tile([C, K9], bf16, tag=f"wk{j}", name=f"wk{j}")
                nc.vector.tensor_scalar(out=wq[:], in0=wtmp[:], scalar1=MAGIC,
                                        scalar2=None, op0=OP.subtract)
                wk.append(wq)
                wmaxg.append(wmr)

            gathx = smallp.tile([n_cores, C], f32, tag="gathx", name="gathx")
            nc.gpsimd.dma_start(gathx[:], ccx_o[:])
            tpx = psump.tile([C, n_cores], f32, tag="ps", name="tpx")
            nc.tensor.transpose(tpx[:], gathx[:], eye8)
            sxch = stile("sxch")   # per-channel global max
            nc.vector.tensor_reduce(out=sxch[:], in_=tpx[:], axis=AX.X,
                                    op=OP.max)
            sx = stile("sx")       # global scalar, replicated
            nc.gpsimd.partition_all_reduce(sx[:], sxch[:], channels=C,
                                           reduce_op=bass_isa.ReduceOp.max)
            sxrec = stile("sxrec")
            nc.vector.reciprocal(sxrec[:], sx[:])
            cx = stile("cx")
            nc.vector.tensor_scalar_mul(cx[:], sxrec[:], QMAX)


            # alpha = s_in*s_w/127^2 and alpha^2, precomputed off the BN
            # critical chain (only need sx/sa1 + wmax)
            def alpha_pre(tag, s_in, wmr, gamma):
                al = stile(f"al_{tag}")
                nc.vector.tensor_tensor(al[:], s_in[:], wmr[:], OP.mult)
                nc.vector.tensor_scalar_mul(al[:], al[:], 1.0 / (QMAX * QMAX))
                nalsq = stile(f"nalsq_{tag}")   # -alpha^2 (for the fused var)
                nc.vector.tensor_tensor(nalsq[:], al[:], al[:], OP.mult)
                nc.vector.tensor_scalar_mul(nalsq[:], nalsq[:], -1.0)
                alg = stile(f"alg_{tag}")       # alpha*gamma, off-critical
                nc.vector.tensor_tensor(alg[:], al[:], gamma, OP.mult)
                return alg, nalsq

            alg1, nalsq1 = alpha_pre("1", sx, wmaxg[0], gamma1)

            # helpers for padded buffers ------------------------------------
            def pad_memset(t):
                # zero the halo (on Pool: keeps the DVE stream clear)
                nc.gpsimd.memset(t[:, 0:WP + 1], 0.0)
                nc.gpsimd.memset(t[:, 1 + (H + 1) * WP:XKLEN], 0.0)
                side = t[:, 1 + WP:1 + (H + 1) * WP].rearrange(
                    "p (r w) -> p r w", w=WP)
                nc.gpsimd.memset(side[:, :, 0:1], 0.0)
                nc.gpsimd.memset(side[:, :, W + 1:W + 2], 0.0)

            def valid_view(t):
                # [C, H, W] view of the valid cells of a padded buffer
                return t[:, WP + 2:WP + 2 + H * WP].rearrange(
                    "p (r w) -> p r w", w=WP)[:, :, 0:W]

            # ---------------- quantize x -> integer bf16 padded -----------
            # pass1 on ACT (cx*x + MAGIC), pass2 on DVE (-MAGIC, bf16 out)
            xk = []
            for n in range(npc):
                xkt = actp.tile([C, XKLEN], bf16, tag=f"act{n}", name=f"act{n}")
                pad_memset(xkt)
                u = zp.tile([C, HWl], f32, tag=f"z{n}", name=f"z{n}")
                nsplit = 8 if n == 0 else 2
                HRq = H // nsplit
                for h in range(nsplit):
                    rsl = slice(h * HRq * W, (h + 1) * HRq * W)
                    nc.scalar.activation(out=u[:, rsl], in_=xs[n][:, rsl],
                                         func=AF.Identity, bias=magic_t[:],
                                         scale=cx[:])
                    nc.vector.tensor_scalar(
                        out=valid_view(xkt)[:, h * HRq:(h + 1) * HRq, :],
                        in0=u[:, rsl].rearrange("p (r w) -> p r w", w=W),
                        scalar1=MAGIC, scalar2=None, op0=OP.subtract)
                xk.append(xkt)

            # ---------------- conv pass helper ----------------------------
            def conv(src_tiles, wq, z_tag, sums, sumsqs, zmaxs=None):
                z_tiles = []
                for n in range(npc):
                    zt = zp.tile([C, HWl], f32, tag=f"{z_tag}{n}", name=f"{z_tag}{n}")
                    zv_all = zt[:].rearrange("p (r w) -> p r w", w=W)
                    for g in range(NCH):
                        ps = psump.tile([C, CF], f32, tag="ps", name="ps")
                        base = 1 + (g * RPC + 1) * WP
                        for kh in range(3):
                            for kw_ in range(3):
                                k = kh * 3 + kw_
                                off = base + (kh - 1) * WP + (kw_ - 1)
                                nc.tensor.matmul(
                                    ps[:],
                                    wq[:, k * C:(k + 1) * C],
                                    src_tiles[n][:, off:off + CF],
                                    start=(k == 0), stop=(k == 8))
                        pv = ps[:].rearrange("p (r w) -> p r w",
                                             w=WP)[:, :, 1:W + 1]
                        zv = zv_all[:, g * RPC:(g + 1) * RPC, :]
                        ci = n * NCH + g
                        nc.scalar.activation(out=zv, in_=pv, func=AF.Copy,
                                             accum_out=sums[:, ci:ci + 1])
                        sq = sqp.tile([C, RPC, W], f32, tag="sq", name="sq")
                        nc.vector.scalar_tensor_tensor(
                            out=sq[:], in0=zv, scalar=1.0, in1=zv,
                            op0=OP.mult, op1=OP.mult,
                            accum_out=sumsqs[:, ci:ci + 1])
                        if zmaxs is not None:
                            nc.vector.tensor_reduce(out=zmaxs[:, ci:ci + 1],
                                                    in_=zv, axis=AX.XY,
                                                    op=OP.max)
                    z_tiles.append(zt)
                return z_tiles

            NCHT = npc * NCH
            sums1 = stile("sums1", NCHT)
            sumsq1 = stile("sumsq1", NCHT)
            zmaxs1 = stile("zmaxs1", NCHT)
            z1 = conv(xk, wk[0], "z", sums1, sumsq1, zmaxs1)

            # ---------------- BN1 stats: one AllGather of [3,C] ------------
            # column-reduce, pre-transpose to rows, 3-descriptor stage DMA
            gin = stile("gin1", 3)
            nc.vector.tensor_reduce(out=gin[:, 0:1], in_=sums1[:], axis=AX.X,
                                    op=OP.add)
            nc.vector.tensor_reduce(out=gin[:, 1:2], in_=sumsq1[:],
                                    axis=AX.X, op=OP.add)
            nc.vector.tensor_reduce(out=gin[:, 2:3], in_=zmaxs1[:],
                                    axis=AX.X, op=OP.max)
            tp1i = psump.tile([3, C], f32, tag="ps", name="tp1i")
            nc.tensor.transpose(tp1i[:], gin[:], eye_sb[:])
            grow1 = smallp.tile([3, C], f32, tag="grow1", name="grow1")
            nc.scalar.activation(out=grow1[:], in_=tp1i[:], func=AF.Copy)

            cc1_i = dramp.tile([3, C], f32, tag="cc1_i", name="cc1_i")
            cc1_o = dramp.tile([n_cores, 3, C], f32, tag="cc1_o", name="cc1_o")
            nc.sync.dma_start(cc1_i[:], grow1[:])
            nc.gpsimd.collective_compute("AllGather", OP.bypass,
                                         replica_groups=groups,
                                         ins=[cc1_i[:].opt()],
                                         outs=[cc1_o[:].opt()])
            # gathered [8, 3*C] on 8 partitions; transpose each stat back to
            # [C, 8] via PE transpose, then reduce across the core axis.
            gath = smallp.tile([n_cores, 3 * C], f32, tag="gath1",
                               name="gath1")
            nc.sync.dma_start(
                gath[:], cc1_o[:].rearrange("r s c -> r (s c)"))
            gv = gath[:].rearrange("r (s c) -> r s c", s=3)
            addg = stile("addg1", 2)   # [sum, sumsq] reduced over cores
            maxg = stile("maxg1", 1)   # zmax reduced over cores
            tp1 = psump.tile([C, 3, n_cores], f32, tag="ps", name="tp1")
            for j in range(3):
                nc.tensor.transpose(tp1[:, j, :], gv[:, j:j + 1, :], eye8)
            nc.vector.tensor_reduce(out=addg[:, 0:2], in_=tp1[:, 0:2, :],
                                    axis=AX.X, op=OP.add)
            nc.vector.tensor_reduce(out=maxg[:, 0:1], in_=tp1[:, 2, :],
                                    axis=AX.X, op=OP.max)

            # ---------------- BN affine constants (per-channel [C,1]) ------
            def bn_affine(tag, addg, alg, nalsq, beta):
                # returns AB: A = alpha*gamma*rsqrt(var+eps), B = beta-mean*A
                moe = stile(f"moe_{tag}", 2)  # [mean, E[z^2]] in one op
                nc.vector.tensor_scalar_mul(moe[:], addg[:, 0:2], 1.0 / M)
                mean_r = moe[:, 0:1]
                nvar = stile(f"nvar_{tag}")   # mean^2 - E[z^2] = -var
                nc.vector.scalar_tensor_tensor(out=nvar[:], in0=mean_r,
                                               scalar=mean_r, in1=moe[:, 1:2],
                                               op0=OP.mult, op1=OP.subtract)
                var_t = stile(f"vart_{tag}")  # (-var)*(-alpha^2)
                nc.vector.tensor_tensor(var_t[:], nvar[:], nalsq[:], OP.mult)
                sd = stile(f"sd_{tag}")
                nc.scalar.activation(out=sd[:], in_=var_t[:], func=AF.Sqrt,
                                     bias=eps_t[:], scale=1.0)
                rsd = stile(f"rsd_{tag}")
                nc.vector.reciprocal(rsd[:], sd[:])
                AB = stile(f"AB_{tag}", 2)
                A, Bc = AB[:, 0:1], AB[:, 1:2]
                nc.vector.tensor_tensor(A, alg[:], rsd[:], OP.mult)
                mA = stile(f"mA_{tag}")
                nc.vector.tensor_tensor(mA[:], mean_r, A, OP.mult)
                nc.vector.tensor_tensor(Bc, beta, mA[:], OP.subtract)
                return AB

            AB1 = bn_affine("1", addg, alg1, nalsq1, beta1)
            A1, B1 = AB1[:, 0:1], AB1[:, 1:2]

            # s_a1 = global max of relu(z*A1+B1); A1>0 (gamma=1) so only the
            # channel maxima matter.
            c1 = stile("cand1")
            nc.vector.scalar_tensor_tensor(out=c1[:], in0=maxg[:, 0:1],
                                           scalar=A1, in1=B1,
                                           op0=OP.mult, op1=OP.add)
            cand = stile("cand")
            nc.vector.tensor_scalar_max(cand[:], c1[:], 0.0)
            sa1 = stile("sa1")
            nc.gpsimd.partition_all_reduce(sa1[:], cand[:], channels=C,
                                           reduce_op=bass_isa.ReduceOp.max)
            sa1rec = stile("sa1rec")
            nc.vector.reciprocal(sa1rec[:], sa1[:])
            q1 = stile("q1")
            nc.vector.tensor_scalar_mul(q1[:], sa1rec[:], QMAX)
            ABq = stile("ABq", 2)
            nc.vector.tensor_scalar(out=ABq[:], in0=AB1[:, 0:2],
                                    scalar1=q1[:], scalar2=None, op0=OP.mult)
            A1q, B1q = ABq[:, 0:1], ABq[:, 1:2]

            alg2, nalsq2 = alpha_pre("2", sa1, wmaxg[1], gamma2)

            # ---------------- apply BN1+ReLU+quantize -> a1k ---------------
            # ACT: relu(z*A+B) in-place; DVE: (+M,-M) dual-op rint into the
            # (still-zero-haloed) xk buffers, bf16.
            a1k = []
            for n in range(npc):
                a1t = xk[n]
                nsplit = 8 if n == 0 else 2
                HR = H // nsplit
                for h in range(nsplit):
                    rsl = slice(h * HR * W, (h + 1) * HR * W)
                    nc.scalar.activation(out=z1[n][:, rsl], in_=z1[n][:, rsl],
                                         func=AF.Relu, bias=B1q,
                                         scale=A1q)
                    nc.vector.tensor_scalar(
                        out=valid_view(a1t)[:, h * HR:(h + 1) * HR, :],
                        in0=z1[n][:, rsl].rearrange("p (r w) -> p r w", w=W),
                        scalar1=MAGIC, scalar2=MAGIC,
                        op0=OP.add, op1=OP.subtract)
                a1k.append(a1t)

            # ---------------- conv2 ---------------------------------------
            sums2 = stile("sums2", NCHT)
            sumsq2 = stile("sumsq2", NCHT)
            z2 = conv(a1k, wk[1], "z", sums2, sumsq2)

            addin2 = stile("addin2", 2)
            nc.vector.tensor_reduce(out=addin2[:, 0:1], in_=sums2[:],
                                    axis=AX.X, op=OP.add)
            nc.vector.tensor_reduce(out=addin2[:, 1:2], in_=sumsq2[:],
                                    axis=AX.X, op=OP.add)
            tp2i = psump.tile([2, C], f32, tag="ps", name="tp2i")
            nc.tensor.transpose(tp2i[:], addin2[:], eye_sb[:])
            grow2 = smallp.tile([2, C], f32, tag="grow2", name="grow2")
            nc.scalar.activation(out=grow2[:], in_=tp2i[:], func=AF.Copy)

            cc2_i = dramp.tile([2, C], f32, tag="cc2_i", name="cc2_i")
            cc2_o = dramp.tile([n_cores, 2, C], f32, tag="cc2_o", name="cc2_o")
            nc.sync.dma_start(cc2_i[:], grow2[:])
            nc.gpsimd.collective_compute("AllGather", OP.bypass,
                                         replica_groups=groups,
                                         ins=[cc2_i[:].opt()],
                                         outs=[cc2_o[:].opt()])
            gath2 = smallp.tile([n_cores, 2 * C], f32, tag="gath2",
                                name="gath2")
            nc.sync.dma_start(
                gath2[:], cc2_o[:].rearrange("r s c -> r (s c)"))
            gv2 = gath2[:].rearrange("r (s c) -> r s c", s=2)
            addg2 = stile("addg2", 2)
            tp2 = psump.tile([C, 2, n_cores], f32, tag="ps", name="tp2")
            for j in range(2):
                nc.tensor.transpose(tp2[:, j, :], gv2[:, j:j + 1, :], eye8)
            nc.vector.tensor_reduce(out=addg2[:, 0:2], in_=tp2[:],
                                    axis=AX.X, op=OP.add)

            AB2 = bn_affine("2", addg2, alg2, nalsq2, beta2)
            A2, B2 = AB2[:, 0:1], AB2[:, 1:2]

            # ---------------- residual + relu + store (bf16) ---------------
            # DVE: t = (A2*z2) + x in-place; ACT: relu(t + B2) -> bf16; DMA.
            HQ = HWl // 4
            for n in range(npc):
                ot = actp.tile([C, XKLEN], bf16, tag=f"act{n}", name=f"o{n}")
                for h in range(4):
                    sl = slice(h * HQ, (h + 1) * HQ)
                    nc.vector.scalar_tensor_tensor(
                        out=z2[n][:, sl], in0=z2[n][:, sl], scalar=A2,
                        in1=xs[n][:, sl], op0=OP.mult, op1=OP.add)
                    nc.scalar.activation(out=ot[:, sl], in_=z2[n][:, sl],
                                         func=AF.Relu, bias=B2, scale=1.0)
                    if h % 2 == 1:
                        hsl = slice((h - 1) * HQ, (h + 1) * HQ)
                        eng = nc.sync if h == 1 else nc.scalar
                        eng.dma_start(out_d[n][:, hsl], ot[:, hsl])

    nc.compile()
    return nc


def prepare_inputs(x, w1, gamma1, beta1, w2, gamma2, beta2,
                   n_cores=N_CORES):
    """Host-side sharding / layout marshaling (no math)."""
    x = np.ascontiguousarray(np.asarray(x, dtype=np.float32))
    B, C, H, W = x.shape
    w1t = np.ascontiguousarray(
        np.asarray(w1, np.float32).transpose(1, 2, 3, 0).reshape(C, 9 * C))
    w2t = np.ascontiguousarray(
        np.asarray(w2, np.float32).transpose(1, 2, 3, 0).reshape(C, 9 * C))
    params = np.ascontiguousarray(np.stack(
        [np.asarray(gamma1, np.float32), np.asarray(beta1, np.float32),
         np.asarray(gamma2, np.float32), np.asarray(beta2, np.float32)],
        axis=1))
    eyec = np.eye(C, dtype=np.float32)
    bf16_np = mybir.dt.np(bf16)
    shards = np.split(x.reshape(B, C, H * W), n_cores, axis=0)
    in_maps = [{"x": np.ascontiguousarray(s),
                "xh": np.ascontiguousarray(s.astype(bf16_np)),
                "w1t": w1t, "w2t": w2t,
                "params": params, "eyec": eyec} for s in shards]
    return in_maps


_module_cache = {}


def _get_module(shape):
    if shape not in _module_cache:
        B, C, H, W = shape
        nc = build_module(B=B, C=C, H=H, W=W)
        nc.m = get_hw_module(nc.m)
        _module_cache[shape] = nc
    return _module_cache[shape]


def run_on_hw(inputs, trace=False, **kwargs):
    x = np.asarray(inputs["x"])
    B, C, H, W = x.shape
    nc = _get_module((B, C, H, W))
    in_maps = prepare_inputs(**inputs)
    res = bass_utils.run_bass_kernel_spmd(
        nc, in_maps, core_ids=list(range(N_CORES)), trace=trace, **kwargs)
    out = np.concatenate([np.asarray(r["out"]).astype(np.float32)
                          for r in res.results], axis=0)
    return out.reshape(B, C, H, W), res


def kernel(**inputs):
    out, _ = run_on_hw(inputs)
    return out


# revision 29
# speedup vs baseline: 1.0693x; 1.0693x over previous
"""Trainium2 Bass kernel for a quantized ResNet BasicBlock:

    out = relu(bn2(qconv2(relu(bn1(qconv1(x))))) + x)

where qconv = 3x3 conv (stride 1, pad 1) on 8-bit symmetric per-tensor
quantized activations/weights (wage-style, forward pass only), and bn is
training-mode BatchNorm2d (batch statistics over N,H,W).

Strategy (8 NeuronCores, data-parallel over batch):
  * Each core gets B/8 samples. Weights/BN params replicated.
  * Quantized values round(v/s*127) are integers in [-127,127] — exact in
    bfloat16 — so each 3x3 conv runs as 9 accumulated bf16 128x128 matmuls
    per output chunk (channels on the partition dim, shifted windows over a
    zero-padded spatial free dim), accumulating exactly in f32 PSUM. The
    (s_in*s_w/127^2) scale is folded into the BN affine transform.
  * All cross-core exchanges are AllGather (cheap mesh forwarding) plus a
    local PE-transpose + DVE reduce; AllReduce (slow mesh compute) is
    avoided.  Collective payloads are staged through a PE pre-transpose so
    each staging DMA is 1-3 descriptors instead of 128 tiny ones.
  * x is loaded with half-sample DMAs alternating between the two HWDGE
    queues (sync/scalar) — descriptor dispatch is the load bottleneck —
    with the local absmax pipelined per half.
  * Engine balance in the convs: PE does matmuls (~59us/conv, the roofline),
    ACT does PSUM->SBUF copies (+channel-sum accumulation) and quantize
    pass 1, DVE does square+sumsq, channel-max and quantize pass 2.
  * gamma=1 => the BN scale A is positive, so the post-BN1-relu quant
    scale needs only channel maxima of the raw conv output (no minima).
  * round-to-nearest-even via the f32 magic-number trick (+1.5*2^23 then
    subtract), matching jnp.round.
  * Output is stored/DMA'd as bf16 (rel-err budget 2e-2; bf16 adds ~2e-3).
"""

import numpy as np

import concourse.bass as bass
import concourse.bacc as bacc
import concourse.mybir as mybir
import concourse.tile as tile
from concourse import bass_isa
from concourse import bass_utils
from concourse.bass_interp import get_hw_module

f32 = mybir.dt.float32
bf16 = mybir.dt.bfloat16
AF = mybir.ActivationFunctionType
OP = mybir.AluOpType
AX = mybir.AxisListType

N_CORES = 8
MAGIC = 12582912.0  # 1.5 * 2^23: (t + MAGIC) - MAGIC == rint(t) for |t| < 2^22
EPS = 1e-5
QMAX = 127.0


def build_module(B=32, C=128, H=56, W=56, n_cores=N_CORES, rows_per_chunk=8):
    npc = B // n_cores          # samples per core
    HWl = H * W
    HH = HWl // 2
    WP = W + 2                  # padded row length
    PADLEN = (H + 2) * WP       # padded image size
    XKLEN = PADLEN + 2          # +1 guard element at each end
    RPC = rows_per_chunk
    assert H % RPC == 0
    NCH = H // RPC              # chunks (row groups) per sample
    CF = RPC * WP               # matmul free size per chunk
    assert CF <= 512
    M = B * HWl                 # BN normalization count (global batch)
    K9 = 9 * C

    nc = bacc.Bacc("TRN2", target_bir_lowering=False, debug=False,
                   num_devices=n_cores)

    x_d = nc.dram_tensor("x", [npc, C, HWl], f32, kind="ExternalInput")
    xh_d = nc.dram_tensor("xh", [npc, C, HWl], bf16, kind="ExternalInput")
    w1_d = nc.dram_tensor("w1t", [C, K9], f32, kind="ExternalInput")
    w2_d = nc.dram_tensor("w2t", [C, K9], f32, kind="ExternalInput")
    par_d = nc.dram_tensor("params", [C, 4], f32, kind="ExternalInput")
    eye_d = nc.dram_tensor("eyec", [C, C], f32, kind="ExternalInput")
    out_d = nc.dram_tensor("out", [npc, C, HWl], bf16, kind="ExternalOutput")

    groups = [list(range(n_cores))]

    with tile.TileContext(nc) as tc:
        with (
            tc.tile_pool(name="const", bufs=1) as constp,
            tc.tile_pool(name="xs", bufs=1) as xsp,
            tc.tile_pool(name="act", bufs=1) as actp,
            tc.tile_pool(name="z", bufs=1) as zp,
            tc.tile_pool(name="small", bufs=1) as smallp,
            tc.tile_pool(name="sq", bufs=4) as sqp,
            tc.tile_pool(name="psum", bufs=8, space="PSUM") as psump,
            tc.tile_pool(name="dram", bufs=1, space="DRAM") as dramp,
        ):
            def stile(tag, cols=1):
                return smallp.tile([C, cols], f32, tag=tag, name=tag)

            # ---------------- absmax from the bf16 copy of x ---------------
            # the quant scale feeds only scale-invariant BN math, so a bf16
            # absmax (half the HBM traffic of f32 — the pre-barrier critical
            # path) is accurate far beyond what the output needs.
            xhs = []
            xmaxs = stile("xmaxs", npc)
            for n in range(npc):
                th = actp.tile([C, XKLEN], bf16, tag=f"act{n}", name=f"xh{n}")
                eng = nc.sync if n % 2 == 0 else nc.scalar
                eng.dma_start(th[:, 0:HWl], xh_d[n])
                nc.vector.tensor_reduce(out=xmaxs[:, n:n + 1],
                                        in_=th[:, 0:HWl], axis=AX.X, op=OP.max,
                                        apply_absolute_value=True)
                xhs.append(th)
            # identity for PE transposes: on sync after the bf16 halves
            # (needed for the s_x staging transpose at ~22us)
            eye_sb = smallp.tile([C, C], f32, tag="eyec", name="eyec")
            nc.sync.dma_start(eye_sb[:], eye_d[:])
            eye8 = eye_sb[0:n_cores, 0:n_cores]

            xmax = stile("xmax")
            nc.vector.tensor_reduce(out=xmax[:], in_=xmaxs[:], axis=AX.X,
                                    op=OP.max)
            # transpose [C,1] -> [1,C] so the staging DMA is ONE descriptor
            tpin = psump.tile([1, C], f32, tag="ps", name="tpin")
            nc.tensor.transpose(tpin[:], xmax[:], eye_sb[:])
            xrow = smallp.tile([1, C], f32, tag="xrow", name="xrow")
            nc.scalar.activation(out=xrow[:], in_=tpin[:], func=AF.Copy)
            # AllGather per-core per-channel maxima, reduce locally.
            # Stage + readback on sync — free once the bf16 halves land
            # (f32 x runs on scalar; SWDGE loses arbitration to busy HWDGE).
            ccx_i = dramp.tile([1, C], f32, tag="ccx_i", name="ccx_i")
            ccx_o = dramp.tile([n_cores, C], f32, tag="ccx_o", name="ccx_o")
            nc.gpsimd.dma_start(ccx_i[:], xrow[:])
            nc.gpsimd.collective_compute("AllGather", OP.bypass,
                                         replica_groups=groups,
                                         ins=[ccx_i[:].opt()],
                                         outs=[ccx_o[:].opt()])

            magic_t = stile("magic")
            nc.gpsimd.memset(magic_t[:], MAGIC)
            eps_t = stile("eps")
            nc.gpsimd.memset(eps_t[:], EPS)
            par_sb = stile("params", 4)
            nc.gpsimd.dma_start(par_sb[:], par_d[:])
            gamma1, beta1 = par_sb[:, 0:1], par_sb[:, 1:2]
            gamma2, beta2 = par_sb[:, 2:3], par_sb[:, 3:4]
            # preload the sqrt-capable activation table off the critical
            # path (a table switch at BN1 costs 1.5us otherwise)
            sqwarm = stile("sqwarm")
            nc.scalar.activation(out=sqwarm[:], in_=eps_t[:], func=AF.Sqrt,
                                 bias=eps_t[:], scale=1.0)

            # ---------------- x (f32): load after the bf16 copy ------------
            # parked past the s_x staging DMA (tile_wait_until) so its 512
            # descriptors don't bury the 1-descriptor stage in the queues
            xs = []
            for n in range(npc):
                t = xsp.tile([C, HWl], f32, tag=f"xs{n}", name=f"xs{n}")
                eng = nc.sync if n % 2 == 0 else nc.scalar
                eng.dma_start(t[:], x_d[n])
                xs.append(t)

            # ---------------- weights: load + quantize to integer bf16 ----
            wk = []     # bf16 integer lhsT weights [C, 9*C]
            wmaxg = []  # replicated per-tensor absmax [C,1]
            for j, w_d in enumerate((w1_d, w2_d)):
                wsb = constp.tile([C, K9], f32, tag=f"wsb{j}", name=f"wsb{j}")
                eng = nc.sync if j == 0 else nc.scalar
                eng.dma_start(wsb[:], w_d[:])
                wm = stile(f"wmax{j}")
                nc.vector.tensor_reduce(out=wm[:], in_=wsb[:], axis=AX.X,
                                        op=OP.max, apply_absolute_value=True)
                wmr = stile(f"wmaxr{j}")
                nc.gpsimd.partition_all_reduce(wmr[:], wm[:], channels=C,
                                               reduce_op=bass_isa.ReduceOp.max)
                wrec = stile(f"wrec{j}")
                nc.vector.reciprocal(wrec[:], wmr[:])
                cw = stile(f"cw{j}")
                nc.vector.tensor_scalar_mul(cw[:], wrec[:], QMAX)
                wtmp = constp.tile([C, K9], f32, tag=f"wtmp{j}", name=f"wtmp{j}")
                nc.scalar.activation(out=wtmp[:], in_=wsb[:], func=AF.Identity,
                                     bias=magic_t[:], scale=cw[:])
                wq = constp.tile([C, K9], bf16, tag=f"wk{j}", name=f"wk{j}")
                nc.vector.tensor_scalar(out=wq[:], in0=wtmp[:], scalar1=MAGIC,
                                        scalar2=None, op0=OP.subtract)
                wk.append(wq)
                wmaxg.append(wmr)

            gathx = smallp.tile([n_cores, C], f32, tag="gathx", name="gathx")
            nc.gpsimd.dma_start(gathx[:], ccx_o[:])
            tpx = psump.tile([C, n_cores], f32, tag="ps", name="tpx")
            nc.tensor.transpose(tpx[:], gathx[:], eye8)
            sxch = stile("sxch")   # per-channel global max
            nc.vector.tensor_reduce(out=sxch[:], in_=tpx[:], axis=AX.X,
                                    op=OP.max)
            sx = stile("sx")       # global scalar, replicated
            nc.gpsimd.partition_all_reduce(sx[:], sxch[:], channels=C,
                                           reduce_op=bass_isa.ReduceOp.max)
            sxrec = stile("sxrec")
            nc.vector.reciprocal(sxrec[:], sx[:])
            cx = stile("cx")
            nc.vector.tensor_scalar_mul(cx[:], sxrec[:], QMAX)


            # alpha = s_in*s_w/127^2 and alpha^2, precomputed off the BN
            # critical chain (only need sx/sa1 + wmax)
            def alpha_pre(tag, s_in, wmr, gamma):
                al = stile(f"al_{tag}")
                nc.vector.tensor_tensor(al[:], s_in[:], wmr[:], OP.mult)
                nc.vector.tensor_scalar_mul(al[:], al[:], 1.0 / (QMAX * QMAX))
                nalsq = stile(f"nalsq_{tag}")   # -alpha^2 (for the fused var)
                nc.vector.tensor_tensor(nalsq[:], al[:], al[:], OP.mult)
                nc.vector.tensor_scalar_mul(nalsq[:], nalsq[:], -1.0)
                alg = stile(f"alg_{tag}")       # alpha*gamma, off-critical
                nc.vector.tensor_tensor(alg[:], al[:], gamma, OP.mult)
                return alg, nalsq

            alg1, nalsq1 = alpha_pre("1", sx, wmaxg[0], gamma1)

            # helpers for padded buffers ------------------------------------
            def pad_memset(t):
                # zero the halo (on Pool: keeps the DVE stream clear)
                nc.gpsimd.memset(t[:, 0:WP + 1], 0.0)
                nc.gpsimd.memset(t[:, 1 + (H + 1) * WP:XKLEN], 0.0)
                side = t[:, 1 + WP:1 + (H + 1) * WP].rearrange(
                    "p (r w) -> p r w", w=WP)
                nc.gpsimd.memset(side[:, :, 0:1], 0.0)
                nc.gpsimd.memset(side[:, :, W + 1:W + 2], 0.0)

            def valid_view(t):
                # [C, H, W] view of the valid cells of a padded buffer
                return t[:, WP + 2:WP + 2 + H * WP].rearrange(
                    "p (r w) -> p r w", w=WP)[:, :, 0:W]

            # ---------------- quantize x -> integer bf16 padded -----------
            # pass1 on ACT (cx*x + MAGIC), pass2 on DVE (-MAGIC, bf16 out)
            xk = []
            for n in range(npc):
                xkt = actp.tile([C, XKLEN], bf16, tag=f"act{n}", name=f"act{n}")
                pad_memset(xkt)
                u = zp.tile([C, HWl], f32, tag=f"z{n}", name=f"z{n}")
                nsplit = 8 if n == 0 else 2
                HRq = H // nsplit
                for h in range(nsplit):
                    rsl = slice(h * HRq * W, (h + 1) * HRq * W)
                    nc.scalar.activation(out=u[:, rsl], in_=xs[n][:, rsl],
                                         func=AF.Identity, bias=magic_t[:],
                                         scale=cx[:])
                    nc.vector.tensor_scalar(
                        out=valid_view(xkt)[:, h * HRq:(h + 1) * HRq, :],
                        in0=u[:, rsl].rearrange("p (r w) -> p r w", w=W),
                        scalar1=MAGIC, scalar2=None, op0=OP.subtract)
                xk.append(xkt)

            # ---------------- conv pass helper ----------------------------
            def conv(src_tiles, wq, z_tag, sums, sumsqs, zmaxs=None):
                z_tiles = []
                for n in range(npc):
                    zt = zp.tile([C, HWl], f32, tag=f"{z_tag}{n}", name=f"{z_tag}{n}")
                    zv_all = zt[:].rearrange("p (r w) -> p r w", w=W)
                    for g in range(NCH):
                        ps = psump.tile([C, CF], f32, tag="ps", name="ps")
                        base = 1 + (g * RPC + 1) * WP
                        for kh in range(3):
                            for kw_ in range(3):
                                k = kh * 3 + kw_
                                off = base + (kh - 1) * WP + (kw_ - 1)
                                nc.tensor.matmul(
                                    ps[:],
                                    wq[:, k * C:(k + 1) * C],
                                    src_tiles[n][:, off:off + CF],
                                    start=(k == 0), stop=(k == 8))
                        pv = ps[:].rearrange("p (r w) -> p r w",
                                             w=WP)[:, :, 1:W + 1]
                        zv = zv_all[:, g * RPC:(g + 1) * RPC, :]
                        ci = n * NCH + g
                        nc.scalar.activation(out=zv, in_=pv, func=AF.Copy,
                                             accum_out=sums[:, ci:ci + 1])
                        sq = sqp.tile([C, RPC, W], f32, tag="sq", name="sq")
                        nc.vector.scalar_tensor_tensor(
                            out=sq[:], in0=zv, scalar=1.0, in1=zv,
                            op0=OP.mult, op1=OP.mult,
                            accum_out=sumsqs[:, ci:ci + 1])
                        if zmaxs is not None:
                            nc.vector.tensor_reduce(out=zmaxs[:, ci:ci + 1],
                                                    in_=zv, axis=AX.XY,
                                                    op=OP.max)
                    z_tiles.append(zt)
                return z_tiles

            NCHT = npc * NCH
            sums1 = stile("sums1", NCHT)
            sumsq1 = stile("sumsq1", NCHT)
            zmaxs1 = stile("zmaxs1", NCHT)
            z1 = conv(xk, wk[0], "z", sums1, sumsq1, zmaxs1)

            # ---------------- BN1 stats: one AllGather of [3,C] ------------
            # column-reduce, pre-transpose to rows, 3-descriptor stage DMA
            gin = stile("gin1", 3)
            nc.vector.tensor_reduce(out=gin[:, 0:1], in_=sums1[:], axis=AX.X,
                                    op=OP.add)
            nc.vector.tensor_reduce(out=gin[:, 1:2], in_=sumsq1[:],
                                    axis=AX.X, op=OP.add)
            nc.vector.tensor_reduce(out=gin[:, 2:3], in_=zmaxs1[:],
                                    axis=AX.X, op=OP.max)
            tp1i = psump.tile([3, C], f32, tag="ps", name="tp1i")
            nc.tensor.transpose(tp1i[:], gin[:], eye_sb[:])
            grow1 = smallp.tile([3, C], f32, tag="grow1", name="grow1")
            nc.scalar.activation(out=grow1[:], in_=tp1i[:], func=AF.Copy)

            cc1_i = dramp.tile([3, C], f32, tag="cc1_i", name="cc1_i")
            cc1_o = dramp.tile([n_cores, 3, C], f32, tag="cc1_o", name="cc1_o")
            nc.sync.dma_start(cc1_i[:], grow1[:])
            nc.gpsimd.collective_compute("AllGather", OP.bypass,
                                         replica_groups=groups,
                                         ins=[cc1_i[:].opt()],
                                         outs=[cc1_o[:].opt()])
            # gathered [8, 3*C] on 8 partitions; transpose each stat back to
            # [C, 8] via PE transpose, then reduce across the core axis.
            gath = smallp.tile([n_cores, 3 * C], f32, tag="gath1",
                               name="gath1")
            nc.sync.dma_start(
                gath[:], cc1_o[:].rearrange("r s c -> r (s c)"))
            gv = gath[:].rearrange("r (s c) -> r s c", s=3)
            addg = stile("addg1", 2)   # [sum, sumsq] reduced over cores
            maxg = stile("maxg1", 1)   # zmax reduced over cores
            red_specs = [(0, addg[:, 0:1], OP.add), (1, addg[:, 1:2], OP.add),
                         (2, maxg[:, 0:1], OP.max)]
            for j, dst, op in red_specs:
                tp = psump.tile([C, n_cores], f32, tag="ps", name="tp")
                nc.tensor.transpose(tp[:], gv[:, j:j + 1, :], eye8)
                nc.vector.tensor_reduce(out=dst, in_=tp[:], axis=AX.X, op=op)

            # ---------------- BN affine constants (per-channel [C,1]) ------
            def bn_affine(tag, addg, alg, nalsq, beta):
                # returns AB: A = alpha*gamma*rsqrt(var+eps), B = beta-mean*A
                moe = stile(f"moe_{tag}", 2)  # [mean, E[z^2]] in one op
                nc.vector.tensor_scalar_mul(moe[:], addg[:, 0:2], 1.0 / M)
                mean_r = moe[:, 0:1]
                nvar = stile(f"nvar_{tag}")   # mean^2 - E[z^2] = -var
                nc.vector.scalar_tensor_tensor(out=nvar[:], in0=mean_r,
                                               scalar=mean_r, in1=moe[:, 1:2],
                                               op0=OP.mult, op1=OP.subtract)
                var_t = stile(f"vart_{tag}")  # (-var)*(-alpha^2)
                nc.vector.tensor_tensor(var_t[:], nvar[:], nalsq[:], OP.mult)
                sd = stile(f"sd_{tag}")
                nc.scalar.activation(out=sd[:], in_=var_t[:], func=AF.Sqrt,
                                     bias=eps_t[:], scale=1.0)
                rsd = stile(f"rsd_{tag}")
                nc.vector.reciprocal(rsd[:], sd[:])
                AB = stile(f"AB_{tag}", 2)
                A, Bc = AB[:, 0:1], AB[:, 1:2]
                nc.vector.tensor_tensor(A, alg[:], rsd[:], OP.mult)
                mA = stile(f"mA_{tag}")
                nc.vector.tensor_tensor(mA[:], mean_r, A, OP.mult)
                nc.vector.tensor_tensor(Bc, beta, mA[:], OP.subtract)
                return AB

            AB1 = bn_affine("1", addg, alg1, nalsq1, beta1)
            A1, B1 = AB1[:, 0:1], AB1[:, 1:2]

            # s_a1 = global max of relu(z*A1+B1); A1>0 (gamma=1) so only the
            # channel maxima matter.
            c1 = stile("cand1")
            nc.vector.scalar_tensor_tensor(out=c1[:], in0=maxg[:, 0:1],
                                           scalar=A1, in1=B1,
                                           op0=OP.mult, op1=OP.add)
            cand = stile("cand")
            nc.vector.tensor_scalar_max(cand[:], c1[:], 0.0)
            sa1 = stile("sa1")
            nc.gpsimd.partition_all_reduce(sa1[:], cand[:], channels=C,
                                           reduce_op=bass_isa.ReduceOp.max)
            sa1rec = stile("sa1rec")
            nc.vector.reciprocal(sa1rec[:], sa1[:])
            q1 = stile("q1")
            nc.vector.tensor_scalar_mul(q1[:], sa1rec[:], QMAX)
            ABq = stile("ABq", 2)
            nc.vector.tensor_scalar(out=ABq[:], in0=AB1[:, 0:2],
                                    scalar1=q1[:], scalar2=None, op0=OP.mult)
            A1q, B1q = ABq[:, 0:1], ABq[:, 1:2]

            alg2, nalsq2 = alpha_pre("2", sa1, wmaxg[1], gamma2)

            # ---------------- apply BN1+ReLU+quantize -> a1k ---------------
            # ACT: relu(z*A+B) in-place; DVE: (+M,-M) dual-op rint into the
            # (still-zero-haloed) xk buffers, bf16.
            a1k = []
            for n in range(npc):
                a1t = xk[n]
                nsplit = 8 if n == 0 else 2
                HR = H // nsplit
                for h in range(nsplit):
                    rsl = slice(h * HR * W, (h + 1) * HR * W)
                    nc.scalar.activation(out=z1[n][:, rsl], in_=z1[n][:, rsl],
                                         func=AF.Relu, bias=B1q,
                                         scale=A1q)
                    nc.vector.tensor_scalar(
                        out=valid_view(a1t)[:, h * HR:(h + 1) * HR, :],
                        in0=z1[n][:, rsl].rearrange("p (r w) -> p r w", w=W),
                        scalar1=MAGIC, scalar2=MAGIC,
                        op0=OP.add, op1=OP.subtract)
                a1k.append(a1t)

            # ---------------- conv2 ---------------------------------------
            sums2 = stile("sums2", NCHT)
            sumsq2 = stile("sumsq2", NCHT)
            z2 = conv(a1k, wk[1], "z", sums2, sumsq2)

            addin2 = stile("addin2", 2)
            nc.vector.tensor_reduce(out=addin2[:, 0:1], in_=sums2[:],
                                    axis=AX.X, op=OP.add)
            nc.vector.tensor_reduce(out=addin2[:, 1:2], in_=sumsq2[:],
                                    axis=AX.X, op=OP.add)
            tp2i = psump.tile([2, C], f32, tag="ps", name="tp2i")
            nc.tensor.transpose(tp2i[:], addin2[:], eye_sb[:])
            grow2 = smallp.tile([2, C], f32, tag="grow2", name="grow2")
            nc.scalar.activation(out=grow2[:], in_=tp2i[:], func=AF.Copy)

            cc2_i = dramp.tile([2, C], f32, tag="cc2_i", name="cc2_i")
            cc2_o = dramp.tile([n_cores, 2, C], f32, tag="cc2_o", name="cc2_o")
            nc.sync.dma_start(cc2_i[:], grow2[:])
            nc.gpsimd.collective_compute("AllGather", OP.bypass,
                                         replica_groups=groups,
                                         ins=[cc2_i[:].opt()],
                                         outs=[cc2_o[:].opt()])
            gath2 = smallp.tile([n_cores, 2 * C], f32, tag="gath2",
                                name="gath2")
            nc.sync.dma_start(
                gath2[:], cc2_o[:].rearrange("r s c -> r (s c)"))
            gv2 = gath2[:].rearrange("r (s c) -> r s c", s=2)
            addg2 = stile("addg2", 2)
            for j in range(2):
                tp = psump.tile([C, n_cores], f32, tag="ps", name="tp")
                nc.tensor.transpose(tp[:], gv2[:, j:j + 1, :], eye8)
                nc.vector.tensor_reduce(out=addg2[:, j:j + 1], in_=tp[:],
                                        axis=AX.X, op=OP.add)

            AB2 = bn_affine("2", addg2, alg2, nalsq2, beta2)
            A2, B2 = AB2[:, 0:1], AB2[:, 1:2]

            # ---------------- residual + relu + store (bf16) ---------------
            # DVE: t = (A2*z2) + x in-place; ACT: relu(t + B2) -> bf16; DMA.
            for n in range(npc):
                ot = actp.tile([C, XKLEN], bf16, tag=f"act{n}", name=f"o{n}")
                for h in range(2):
                    sl = slice(h * HH, (h + 1) * HH)
                    nc.vector.scalar_tensor_tensor(
                        out=z2[n][:, sl], in0=z2[n][:, sl], scalar=A2,
                        in1=xs[n][:, sl], op0=OP.mult, op1=OP.add)
                    nc.scalar.activation(out=ot[:, sl], in_=z2[n][:, sl],
                                         func=AF.Relu, bias=B2, scale=1.0)
                # one full-sample DMA: half the descriptor count
                eng = nc.sync if n % 2 == 0 else nc.scalar
                eng.dma_start(out_d[n], ot[:, 0:HWl])

    nc.compile()
    return nc


def prepare_inputs(x, w1, gamma1, beta1, w2, gamma2, beta2,
                   n_cores=N_CORES):
    """Host-side sharding / layout marshaling (no math)."""
    x = np.ascontiguousarray(np.asarray(x, dtype=np.float32))
    B, C, H, W = x.shape
    w1t = np.ascontiguousarray(
        np.asarray(w1, np.float32).transpose(1, 2, 3, 0).reshape(C, 9 * C))
    w2t = np.ascontiguousarray(
        np.asarray(w2, np.float32).transpose(1, 2, 3, 0).reshape(C, 9 * C))
    params = np.ascontiguousarray(np.stack(
        [np.asarray(gamma1, np.float32), np.asarray(beta1, np.float32),
         np.asarray(gamma2, np.float32), np.asarray(beta2, np.float32)],
        axis=1))
    eyec = np.eye(C, dtype=np.float32)
    bf16_np = mybir.dt.np(bf16)
    shards = np.split(x.reshape(B, C, H * W), n_cores, axis=0)
    in_maps = [{"x": np.ascontiguousarray(s),
                "xh": np.ascontiguousarray(s.astype(bf16_np)),
                "w1t": w1t, "w2t": w2t,
                "params": params, "eyec": eyec} for s in shards]
    return in_maps


_module_cache = {}


def _get_module(shape):
    if shape not in _module_cache:
        B, C, H, W = shape
        nc = build_module(B=B, C=C, H=H, W=W)
        nc.m = get_hw_module(nc.m)
        _module_cache[shape] = nc
    return _module_cache[shape]


def run_on_hw(inputs, trace=False, **kwargs):
    x = np.asarray(inputs["x"])
    B, C, H, W = x.shape
    nc = _get_module((B, C, H, W))
    in_maps = prepare_inputs(**inputs)
    res = bass_utils.run_bass_kernel_spmd(
        nc, in_maps, core_ids=list(range(N_CORES)), trace=trace, **kwargs)
    out = np.concatenate([np.asarray(r["out"]).astype(np.float32)
                          for r in res.results], axis=0)
    return out.reshape(B, C, H, W), res


def kernel(**inputs):
    out, _ = run_on_hw(inputs)
    return out


# revision 30
# speedup vs baseline: 1.1454x; 1.0712x over previous
"""Trainium2 Bass kernel for a quantized ResNet BasicBlock:

    out = relu(bn2(qconv2(relu(bn1(qconv1(x))))) + x)

where qconv = 3x3 conv (stride 1, pad 1) on 8-bit symmetric per-tensor
quantized activations/weights (wage-style, forward pass only), and bn is
training-mode BatchNorm2d (batch statistics over N,H,W).

Strategy (8 NeuronCores, data-parallel over batch):
  * Each core gets B/8 samples. Weights/BN params replicated.
  * Quantized values round(v/s*127) are integers in [-127,127] — exact in
    bfloat16 — so each 3x3 conv runs as 9 accumulated bf16 128x128 matmuls
    per output chunk (channels on the partition dim, shifted windows over a
    zero-padded spatial free dim), accumulating exactly in f32 PSUM. The
    (s_in*s_w/127^2) scale is folded into the BN affine transform.
  * All cross-core exchanges are AllGather (cheap mesh forwarding) plus a
    local PE-transpose + DVE reduce; AllReduce (slow mesh compute) is
    avoided.  Collective payloads are staged through a PE pre-transpose so
    each staging DMA is 1-3 descriptors instead of 128 tiny ones.
  * x is loaded with half-sample DMAs alternating between the two HWDGE
    queues (sync/scalar) — descriptor dispatch is the load bottleneck —
    with the local absmax pipelined per half.
  * Engine balance in the convs: PE does matmuls (~59us/conv, the roofline),
    ACT does PSUM->SBUF copies (+channel-sum accumulation) and quantize
    pass 1, DVE does square+sumsq, channel-max and quantize pass 2.
  * gamma=1 => the BN scale A is positive, so the post-BN1-relu quant
    scale needs only channel maxima of the raw conv output (no minima).
  * round-to-nearest-even via the f32 magic-number trick (+1.5*2^23 then
    subtract), matching jnp.round.
  * Output is stored/DMA'd as bf16 (rel-err budget 2e-2; bf16 adds ~2e-3).
"""

import numpy as np

import concourse.bass as bass
import concourse.bacc as bacc
import concourse.mybir as mybir
import concourse.tile as tile
from concourse import bass_isa
from concourse import bass_utils
from concourse.bass_interp import get_hw_module

f32 = mybir.dt.float32
bf16 = mybir.dt.bfloat16
AF = mybir.ActivationFunctionType
OP = mybir.AluOpType
AX = mybir.AxisListType

N_CORES = 8
MAGIC = 12582912.0  # 1.5 * 2^23: (t + MAGIC) - MAGIC == rint(t) for |t| < 2^22
EPS = 1e-5
QMAX = 127.0


def build_module(B=32, C=128, H=56, W=56, n_cores=N_CORES, rows_per_chunk=8):
    npc = B // n_cores          # samples per core
    HWl = H * W
    HH = HWl // 2
    WP = W + 2                  # padded row length
    PADLEN = (H + 2) * WP       # padded image size
    XKLEN = PADLEN + 2          # +1 guard element at each end
    RPC = rows_per_chunk
    assert H % RPC == 0
    NCH = H // RPC              # chunks (row groups) per sample
    CF = RPC * WP               # matmul free size per chunk
    assert CF <= 512
    M = B * HWl                 # BN normalization count (global batch)
    K9 = 9 * C

    nc = bacc.Bacc("TRN2", target_bir_lowering=False, debug=False,
                   num_devices=n_cores)

    xh_d = nc.dram_tensor("xh", [npc, C, HWl], bf16, kind="ExternalInput")
    w1_d = nc.dram_tensor("w1t", [C, K9], f32, kind="ExternalInput")
    w2_d = nc.dram_tensor("w2t", [C, K9], f32, kind="ExternalInput")
    par_d = nc.dram_tensor("params", [C, 4], f32, kind="ExternalInput")
    eye_d = nc.dram_tensor("eyec", [C, C], f32, kind="ExternalInput")
    out_d = nc.dram_tensor("out", [npc, C, HWl], bf16, kind="ExternalOutput")

    groups = [list(range(n_cores))]

    with tile.TileContext(nc) as tc:
        with (
            tc.tile_pool(name="const", bufs=1) as constp,
            tc.tile_pool(name="xs", bufs=1) as xsp,
            tc.tile_pool(name="act", bufs=1) as actp,
            tc.tile_pool(name="z", bufs=1) as zp,
            tc.tile_pool(name="small", bufs=1) as smallp,
            tc.tile_pool(name="sq", bufs=4) as sqp,
            tc.tile_pool(name="psum", bufs=8, space="PSUM") as psump,
            tc.tile_pool(name="dram", bufs=1, space="DRAM") as dramp,
        ):
            def stile(tag, cols=1):
                return smallp.tile([C, cols], f32, tag=tag, name=tag)

            # ---------------- absmax from the bf16 copy of x ---------------
            # the quant scale feeds only scale-invariant BN math, so a bf16
            # absmax (half the HBM traffic of f32 — the pre-barrier critical
            # path) is accurate far beyond what the output needs.
            xhs = []
            xmaxs = stile("xmaxs", npc)
            for n in range(npc):
                th = xsp.tile([C, HWl], bf16, tag=f"xh{n}", name=f"xh{n}")
                eng = nc.sync if n % 2 == 0 else nc.scalar
                eng.dma_start(th[:], xh_d[n])
                nc.vector.tensor_reduce(out=xmaxs[:, n:n + 1],
                                        in_=th[:], axis=AX.X, op=OP.max,
                                        apply_absolute_value=True)
                xhs.append(th)
            # identity for PE transposes: on sync after the bf16 halves
            # (needed for the s_x staging transpose at ~22us)
            eye_sb = smallp.tile([C, C], f32, tag="eyec", name="eyec")
            nc.sync.dma_start(eye_sb[:], eye_d[:])
            eye8 = eye_sb[0:n_cores, 0:n_cores]

            xmax = stile("xmax")
            nc.vector.tensor_reduce(out=xmax[:], in_=xmaxs[:], axis=AX.X,
                                    op=OP.max)
            # transpose [C,1] -> [1,C] so the staging DMA is ONE descriptor
            tpin = psump.tile([1, C], f32, tag="ps", name="tpin")
            nc.tensor.transpose(tpin[:], xmax[:], eye_sb[:])
            xrow = smallp.tile([1, C], f32, tag="xrow", name="xrow")
            nc.scalar.activation(out=xrow[:], in_=tpin[:], func=AF.Copy)
            # AllGather per-core per-channel maxima, reduce locally.
            # Stage + readback on sync — free once the bf16 halves land
            # (f32 x runs on scalar; SWDGE loses arbitration to busy HWDGE).
            ccx_i = dramp.tile([1, C], f32, tag="ccx_i", name="ccx_i")
            ccx_o = dramp.tile([n_cores, C], f32, tag="ccx_o", name="ccx_o")
            nc.gpsimd.dma_start(ccx_i[:], xrow[:])
            nc.gpsimd.collective_compute("AllGather", OP.bypass,
                                         replica_groups=groups,
                                         ins=[ccx_i[:].opt()],
                                         outs=[ccx_o[:].opt()])

            magic_t = stile("magic")
            nc.gpsimd.memset(magic_t[:], MAGIC)
            eps_t = stile("eps")
            nc.gpsimd.memset(eps_t[:], EPS)
            par_sb = stile("params", 4)
            nc.gpsimd.dma_start(par_sb[:], par_d[:])
            gamma1, beta1 = par_sb[:, 0:1], par_sb[:, 1:2]
            gamma2, beta2 = par_sb[:, 2:3], par_sb[:, 3:4]
            # preload the sqrt-capable activation table off the critical
            # path (a table switch at BN1 costs 1.5us otherwise)
            sqwarm = stile("sqwarm")
            nc.scalar.activation(out=sqwarm[:], in_=eps_t[:], func=AF.Sqrt,
                                 bias=eps_t[:], scale=1.0)

            # ---------------- weights: load + quantize to integer bf16 ----
            wk = []     # bf16 integer lhsT weights [C, 9*C]
            wmaxg = []  # replicated per-tensor absmax [C,1]
            for j, w_d in enumerate((w1_d, w2_d)):
                wsb = constp.tile([C, K9], f32, tag=f"wsb{j}", name=f"wsb{j}")
                eng = nc.sync if j == 0 else nc.scalar
                eng.dma_start(wsb[:], w_d[:])
                wm = stile(f"wmax{j}")
                nc.vector.tensor_reduce(out=wm[:], in_=wsb[:], axis=AX.X,
                                        op=OP.max, apply_absolute_value=True)
                wmr = stile(f"wmaxr{j}")
                nc.gpsimd.partition_all_reduce(wmr[:], wm[:], channels=C,
                                               reduce_op=bass_isa.ReduceOp.max)
                wrec = stile(f"wrec{j}")
                nc.vector.reciprocal(wrec[:], wmr[:])
                cw = stile(f"cw{j}")
                nc.vector.tensor_scalar_mul(cw[:], wrec[:], QMAX)
                wtmp = constp.tile([C, K9], f32, tag=f"wtmp{j}", name=f"wtmp{j}")
                nc.scalar.activation(out=wtmp[:], in_=wsb[:], func=AF.Identity,
                                     bias=magic_t[:], scale=cw[:])
                wq = constp.tile([C, K9], bf16, tag=f"wk{j}", name=f"wk{j}")
                nc.vector.tensor_scalar(out=wq[:], in0=wtmp[:], scalar1=MAGIC,
                                        scalar2=None, op0=OP.subtract)
                wk.append(wq)
                wmaxg.append(wmr)

            gathx = smallp.tile([n_cores, C], f32, tag="gathx", name="gathx")
            nc.gpsimd.dma_start(gathx[:], ccx_o[:])
            tpx = psump.tile([C, n_cores], f32, tag="ps", name="tpx")
            nc.tensor.transpose(tpx[:], gathx[:], eye8)
            sxch = stile("sxch")   # per-channel global max
            nc.vector.tensor_reduce(out=sxch[:], in_=tpx[:], axis=AX.X,
                                    op=OP.max)
            sx = stile("sx")       # global scalar, replicated
            nc.gpsimd.partition_all_reduce(sx[:], sxch[:], channels=C,
                                           reduce_op=bass_isa.ReduceOp.max)
            sxrec = stile("sxrec")
            nc.vector.reciprocal(sxrec[:], sx[:])
            cx = stile("cx")
            nc.vector.tensor_scalar_mul(cx[:], sxrec[:], QMAX)


            # alpha = s_in*s_w/127^2 and alpha^2, precomputed off the BN
            # critical chain (only need sx/sa1 + wmax)
            def alpha_pre(tag, s_in, wmr, gamma):
                al = stile(f"al_{tag}")
                nc.vector.tensor_tensor(al[:], s_in[:], wmr[:], OP.mult)
                nc.vector.tensor_scalar_mul(al[:], al[:], 1.0 / (QMAX * QMAX))
                nalsq = stile(f"nalsq_{tag}")   # -alpha^2 (for the fused var)
                nc.vector.tensor_tensor(nalsq[:], al[:], al[:], OP.mult)
                nc.vector.tensor_scalar_mul(nalsq[:], nalsq[:], -1.0)
                alg = stile(f"alg_{tag}")       # alpha*gamma, off-critical
                nc.vector.tensor_tensor(alg[:], al[:], gamma, OP.mult)
                return alg, nalsq

            alg1, nalsq1 = alpha_pre("1", sx, wmaxg[0], gamma1)

            # helpers for padded buffers ------------------------------------
            def pad_memset(t):
                # zero the halo (on Pool: keeps the DVE stream clear)
                nc.gpsimd.memset(t[:, 0:WP + 1], 0.0)
                nc.gpsimd.memset(t[:, 1 + (H + 1) * WP:XKLEN], 0.0)
                side = t[:, 1 + WP:1 + (H + 1) * WP].rearrange(
                    "p (r w) -> p r w", w=WP)
                nc.gpsimd.memset(side[:, :, 0:1], 0.0)
                nc.gpsimd.memset(side[:, :, W + 1:W + 2], 0.0)

            def valid_view(t):
                # [C, H, W] view of the valid cells of a padded buffer
                return t[:, WP + 2:WP + 2 + H * WP].rearrange(
                    "p (r w) -> p r w", w=WP)[:, :, 0:W]

            # ---------------- quantize x -> integer bf16 padded -----------
            # pass1 on ACT (cx*x + MAGIC), pass2 on DVE (-MAGIC, bf16 out)
            xk = []
            for n in range(npc):
                xkt = actp.tile([C, XKLEN], bf16, tag=f"act{n}", name=f"act{n}")
                pad_memset(xkt)
                u = zp.tile([C, HWl], f32, tag=f"z{n}", name=f"z{n}")
                nsplit = 8 if n == 0 else 2
                HRq = H // nsplit
                for h in range(nsplit):
                    rsl = slice(h * HRq * W, (h + 1) * HRq * W)
                    nc.scalar.activation(out=u[:, rsl], in_=xhs[n][:, rsl],
                                         func=AF.Identity, bias=magic_t[:],
                                         scale=cx[:])
                    nc.vector.tensor_scalar(
                        out=valid_view(xkt)[:, h * HRq:(h + 1) * HRq, :],
                        in0=u[:, rsl].rearrange("p (r w) -> p r w", w=W),
                        scalar1=MAGIC, scalar2=None, op0=OP.subtract)
                xk.append(xkt)

            # ---------------- conv pass helper ----------------------------
            def conv(src_tiles, wq, z_tag, sums, sumsqs, zmaxs=None):
                z_tiles = []
                for n in range(npc):
                    zt = zp.tile([C, HWl], f32, tag=f"{z_tag}{n}", name=f"{z_tag}{n}")
                    zv_all = zt[:].rearrange("p (r w) -> p r w", w=W)
                    for g in range(NCH):
                        ps = psump.tile([C, CF], f32, tag="ps", name="ps")
                        base = 1 + (g * RPC + 1) * WP
                        for kh in range(3):
                            for kw_ in range(3):
                                k = kh * 3 + kw_
                                off = base + (kh - 1) * WP + (kw_ - 1)
                                nc.tensor.matmul(
                                    ps[:],
                                    wq[:, k * C:(k + 1) * C],
                                    src_tiles[n][:, off:off + CF],
                                    start=(k == 0), stop=(k == 8))
                        pv = ps[:].rearrange("p (r w) -> p r w",
                                             w=WP)[:, :, 1:W + 1]
                        zv = zv_all[:, g * RPC:(g + 1) * RPC, :]
                        ci = n * NCH + g
                        nc.scalar.activation(out=zv, in_=pv, func=AF.Copy,
                                             accum_out=sums[:, ci:ci + 1])
                        sq = sqp.tile([C, RPC, W], f32, tag="sq", name="sq")
                        nc.vector.scalar_tensor_tensor(
                            out=sq[:], in0=zv, scalar=1.0, in1=zv,
                            op0=OP.mult, op1=OP.mult,
                            accum_out=sumsqs[:, ci:ci + 1])
                        if zmaxs is not None:
                            nc.vector.tensor_reduce(out=zmaxs[:, ci:ci + 1],
                                                    in_=zv, axis=AX.XY,
                                                    op=OP.max)
                    z_tiles.append(zt)
                return z_tiles

            NCHT = npc * NCH
            sums1 = stile("sums1", NCHT)
            sumsq1 = stile("sumsq1", NCHT)
            zmaxs1 = stile("zmaxs1", NCHT)
            z1 = conv(xk, wk[0], "z", sums1, sumsq1, zmaxs1)

            # ---------------- BN1 stats: one AllGather of [3,C] ------------
            # column-reduce, pre-transpose to rows, 3-descriptor stage DMA
            gin = stile("gin1", 3)
            nc.vector.tensor_reduce(out=gin[:, 0:1], in_=sums1[:], axis=AX.X,
                                    op=OP.add)
            nc.vector.tensor_reduce(out=gin[:, 1:2], in_=sumsq1[:],
                                    axis=AX.X, op=OP.add)
            nc.vector.tensor_reduce(out=gin[:, 2:3], in_=zmaxs1[:],
                                    axis=AX.X, op=OP.max)
            tp1i = psump.tile([3, C], f32, tag="ps", name="tp1i")
            nc.tensor.transpose(tp1i[:], gin[:], eye_sb[:])
            grow1 = smallp.tile([3, C], f32, tag="grow1", name="grow1")
            nc.scalar.activation(out=grow1[:], in_=tp1i[:], func=AF.Copy)

            cc1_i = dramp.tile([3, C], f32, tag="cc1_i", name="cc1_i")
            cc1_o = dramp.tile([n_cores, 3, C], f32, tag="cc1_o", name="cc1_o")
            nc.sync.dma_start(cc1_i[:], grow1[:])
            nc.gpsimd.collective_compute("AllGather", OP.bypass,
                                         replica_groups=groups,
                                         ins=[cc1_i[:].opt()],
                                         outs=[cc1_o[:].opt()])
            # gathered [8, 3*C] on 8 partitions; transpose each stat back to
            # [C, 8] via PE transpose, then reduce across the core axis.
            gath = smallp.tile([n_cores, 3 * C], f32, tag="gath1",
                               name="gath1")
            nc.sync.dma_start(
                gath[:], cc1_o[:].rearrange("r s c -> r (s c)"))
            gv = gath[:].rearrange("r (s c) -> r s c", s=3)
            addg = stile("addg1", 2)   # [sum, sumsq] reduced over cores
            maxg = stile("maxg1", 1)   # zmax reduced over cores
            red_specs = [(0, addg[:, 0:1], OP.add), (1, addg[:, 1:2], OP.add),
                         (2, maxg[:, 0:1], OP.max)]
            for j, dst, op in red_specs:
                tp = psump.tile([C, n_cores], f32, tag="ps", name="tp")
                nc.tensor.transpose(tp[:], gv[:, j:j + 1, :], eye8)
                nc.vector.tensor_reduce(out=dst, in_=tp[:], axis=AX.X, op=op)

            # ---------------- BN affine constants (per-channel [C,1]) ------
            def bn_affine(tag, addg, alg, nalsq, beta):
                # returns AB: A = alpha*gamma*rsqrt(var+eps), B = beta-mean*A
                moe = stile(f"moe_{tag}", 2)  # [mean, E[z^2]] in one op
                nc.vector.tensor_scalar_mul(moe[:], addg[:, 0:2], 1.0 / M)
                mean_r = moe[:, 0:1]
                nvar = stile(f"nvar_{tag}")   # mean^2 - E[z^2] = -var
                nc.vector.scalar_tensor_tensor(out=nvar[:], in0=mean_r,
                                               scalar=mean_r, in1=moe[:, 1:2],
                                               op0=OP.mult, op1=OP.subtract)
                var_t = stile(f"vart_{tag}")  # (-var)*(-alpha^2)
                nc.vector.tensor_tensor(var_t[:], nvar[:], nalsq[:], OP.mult)
                sd = stile(f"sd_{tag}")
                nc.scalar.activation(out=sd[:], in_=var_t[:], func=AF.Sqrt,
                                     bias=eps_t[:], scale=1.0)
                rsd = stile(f"rsd_{tag}")
                nc.vector.reciprocal(rsd[:], sd[:])
                AB = stile(f"AB_{tag}", 2)
                A, Bc = AB[:, 0:1], AB[:, 1:2]
                nc.vector.tensor_tensor(A, alg[:], rsd[:], OP.mult)
                mA = stile(f"mA_{tag}")
                nc.vector.tensor_tensor(mA[:], mean_r, A, OP.mult)
                nc.vector.tensor_tensor(Bc, beta, mA[:], OP.subtract)
                return AB

            AB1 = bn_affine("1", addg, alg1, nalsq1, beta1)
            A1, B1 = AB1[:, 0:1], AB1[:, 1:2]

            # s_a1 = global max of relu(z*A1+B1); A1>0 (gamma=1) so only the
            # channel maxima matter.
            c1 = stile("cand1")
            nc.vector.scalar_tensor_tensor(out=c1[:], in0=maxg[:, 0:1],
                                           scalar=A1, in1=B1,
                                           op0=OP.mult, op1=OP.add)
            cand = stile("cand")
            nc.vector.tensor_scalar_max(cand[:], c1[:], 0.0)
            sa1 = stile("sa1")
            nc.gpsimd.partition_all_reduce(sa1[:], cand[:], channels=C,
                                           reduce_op=bass_isa.ReduceOp.max)
            sa1rec = stile("sa1rec")
            nc.vector.reciprocal(sa1rec[:], sa1[:])
            q1 = stile("q1")
            nc.vector.tensor_scalar_mul(q1[:], sa1rec[:], QMAX)
            ABq = stile("ABq", 2)
            nc.vector.tensor_scalar(out=ABq[:], in0=AB1[:, 0:2],
                                    scalar1=q1[:], scalar2=None, op0=OP.mult)
            A1q, B1q = ABq[:, 0:1], ABq[:, 1:2]

            alg2, nalsq2 = alpha_pre("2", sa1, wmaxg[1], gamma2)

            # ---------------- apply BN1+ReLU+quantize -> a1k ---------------
            # ACT: relu(z*A+B) in-place; DVE: (+M,-M) dual-op rint into the
            # (still-zero-haloed) xk buffers, bf16.
            a1k = []
            for n in range(npc):
                a1t = xk[n]
                nsplit = 8 if n == 0 else 2
                HR = H // nsplit
                for h in range(nsplit):
                    rsl = slice(h * HR * W, (h + 1) * HR * W)
                    nc.scalar.activation(out=z1[n][:, rsl], in_=z1[n][:, rsl],
                                         func=AF.Relu, bias=B1q,
                                         scale=A1q)
                    nc.vector.tensor_scalar(
                        out=valid_view(a1t)[:, h * HR:(h + 1) * HR, :],
                        in0=z1[n][:, rsl].rearrange("p (r w) -> p r w", w=W),
                        scalar1=MAGIC, scalar2=MAGIC,
                        op0=OP.add, op1=OP.subtract)
                a1k.append(a1t)

            # ---------------- conv2 ---------------------------------------
            sums2 = stile("sums2", NCHT)
            sumsq2 = stile("sumsq2", NCHT)
            z2 = conv(a1k, wk[1], "z", sums2, sumsq2)

            addin2 = stile("addin2", 2)
            nc.vector.tensor_reduce(out=addin2[:, 0:1], in_=sums2[:],
                                    axis=AX.X, op=OP.add)
            nc.vector.tensor_reduce(out=addin2[:, 1:2], in_=sumsq2[:],
                                    axis=AX.X, op=OP.add)
            tp2i = psump.tile([2, C], f32, tag="ps", name="tp2i")
            nc.tensor.transpose(tp2i[:], addin2[:], eye_sb[:])
            grow2 = smallp.tile([2, C], f32, tag="grow2", name="grow2")
            nc.scalar.activation(out=grow2[:], in_=tp2i[:], func=AF.Copy)

            cc2_i = dramp.tile([2, C], f32, tag="cc2_i", name="cc2_i")
            cc2_o = dramp.tile([n_cores, 2, C], f32, tag="cc2_o", name="cc2_o")
            nc.sync.dma_start(cc2_i[:], grow2[:])
            nc.gpsimd.collective_compute("AllGather", OP.bypass,
                                         replica_groups=groups,
                                         ins=[cc2_i[:].opt()],
                                         outs=[cc2_o[:].opt()])
            gath2 = smallp.tile([n_cores, 2 * C], f32, tag="gath2",
                                name="gath2")
            nc.sync.dma_start(
                gath2[:], cc2_o[:].rearrange("r s c -> r (s c)"))
            gv2 = gath2[:].rearrange("r (s c) -> r s c", s=2)
            addg2 = stile("addg2", 2)
            for j in range(2):
                tp = psump.tile([C, n_cores], f32, tag="ps", name="tp")
                nc.tensor.transpose(tp[:], gv2[:, j:j + 1, :], eye8)
                nc.vector.tensor_reduce(out=addg2[:, j:j + 1], in_=tp[:],
                                        axis=AX.X, op=OP.add)

            AB2 = bn_affine("2", addg2, alg2, nalsq2, beta2)
            A2, B2 = AB2[:, 0:1], AB2[:, 1:2]

            # ---------------- residual + relu + store (bf16) ---------------
            # DVE: t = (A2*z2) + x in-place; ACT: relu(t + B2) -> bf16; DMA.
            for n in range(npc):
                ot = actp.tile([C, XKLEN], bf16, tag=f"act{n}", name=f"o{n}")
                for h in range(2):
                    sl = slice(h * HH, (h + 1) * HH)
                    nc.vector.scalar_tensor_tensor(
                        out=z2[n][:, sl], in0=z2[n][:, sl], scalar=A2,
                        in1=xhs[n][:, sl], op0=OP.mult, op1=OP.add)
                    nc.scalar.activation(out=ot[:, sl], in_=z2[n][:, sl],
                                         func=AF.Relu, bias=B2, scale=1.0)
                # one full-sample DMA: half the descriptor count
                eng = nc.sync if n % 2 == 0 else nc.scalar
                eng.dma_start(out_d[n], ot[:, 0:HWl])

    nc.compile()
    return nc


def prepare_inputs(x, w1, gamma1, beta1, w2, gamma2, beta2,
                   n_cores=N_CORES):
    """Host-side sharding / layout marshaling (no math)."""
    x = np.ascontiguousarray(np.asarray(x, dtype=np.float32))
    B, C, H, W = x.shape
    w1t = np.ascontiguousarray(
        np.asarray(w1, np.float32).transpose(1, 2, 3, 0).reshape(C, 9 * C))
    w2t = np.ascontiguousarray(
        np.asarray(w2, np.float32).transpose(1, 2, 3, 0).reshape(C, 9 * C))
    params = np.ascontiguousarray(np.stack(
        [np.asarray(gamma1, np.float32), np.asarray(beta1, np.float32),
         np.asarray(gamma2, np.float32), np.asarray(beta2, np.float32)],
        axis=1))
    eyec = np.eye(C, dtype=np.float32)
    bf16_np = mybir.dt.np(bf16)
    shards = np.split(x.reshape(B, C, H * W), n_cores, axis=0)
    in_maps = [{"xh": np.ascontiguousarray(s.astype(bf16_np)),
                "w1t": w1t, "w2t": w2t,
                "params": params, "eyec": eyec} for s in shards]
    return in_maps


_module_cache = {}


def _get_module(shape):
    if shape not in _module_cache:
        B, C, H, W = shape
        nc = build_module(B=B, C=C, H=H, W=W)
        nc.m = get_hw_module(nc.m)
        _module_cache[shape] = nc
    return _module_cache[shape]


def run_on_hw(inputs, trace=False, **kwargs):
    x = np.asarray(inputs["x"])
    B, C, H, W = x.shape
    nc = _get_module((B, C, H, W))
    in_maps = prepare_inputs(**inputs)
    res = bass_utils.run_bass_kernel_spmd(
        nc, in_maps, core_ids=list(range(N_CORES)), trace=trace, **kwargs)
    out = np.concatenate([np.asarray(r["out"]).astype(np.float32)
                          for r in res.results], axis=0)
    return out.reshape(B, C, H, W), res


def kernel(**inputs):
    out, _ = run_on_hw(inputs)
    return out
